# revision 26
# baseline (speedup 1.0000x reference)
"""Trainium2 Bass kernel for BinaryMLP:
    h = relu(x @ sign(w1).T + b1); h = relu(h @ sign(w2).T + b2);
    h = relu(h @ sign(w3).T + b3); y = h @ w4.T + b4

Data-parallel over 8 NeuronCores: batch 65536 -> 8192 rows/core, weights
replicated. On-device dataflow is feature-major ("transposed"): activations
live in SBUF as [feature_partition, batch_free] so every layer's contraction
dim (the feature/hidden dim) is the PE partition dim. The host only slices
the batch, transposes/casts for layout, and concatenates the result back.

Compute is bf16 on the tensor engine (binary +-1 weights are exact in bf16;
PSUM accumulates fp32; x is rounded to bf16 host-side — identical numerics
to an on-device cast). Binarization (sign of the latent fp32 weights) runs
on the scalar engine; bias+relu runs on the scalar engine reading PSUM and
writing bf16 back to SBUF. Each batch chunk/weight matrix loads with one
batched DMA (packets spread over all 16 DMA engines); the sync-queue
dispatch order doubles as the HBM priority order in the prologue. Chunks
are processed in interleaved pairs so the PE can run chunk c+1's fc1 while
chunk c's relu results are still in flight (kills the layer-boundary
stalls; peak PSUM use stays at 8 banks).
"""

import numpy as np

N_CORES = 8
F_IN = 784  # input features: 7 k-tiles of 112
K1 = 112
NK1 = 7
H = 512  # hidden width: 4 k-tiles / m-tiles of 128
NKH = 4
N_OUT = 10
CHUNK = 512  # batch columns per moving-operand chunk


def build_nc(b_shard: int, num_devices: int = N_CORES, chunk: int = CHUNK):
    """Build + compile the per-core Bass program for a batch shard of
    b_shard columns. Every core runs the identical program."""
    import concourse.bacc as bacc
    import concourse.mybir as mybir
    import concourse.tile as tile

    f32 = mybir.dt.float32
    bf16 = mybir.dt.bfloat16
    ActFn = mybir.ActivationFunctionType

    # chunk schedule: first pair at half width so the PE can start on a
    # quarter of the data (same per-element PE efficiency at N=256), then
    # full-width chunks
    small = chunk // 2
    assert (b_shard - 2 * small) % chunk == 0
    chunks = [(0, small), (small, small)]
    off = 2 * small
    while off < b_shard:
        chunks.append((off, chunk))
        off += chunk

    nc = bacc.Bacc(
        "TRN2", target_bir_lowering=False, debug=False, num_devices=num_devices
    )

    xT = nc.dram_tensor("xT", [F_IN, b_shard], bf16, kind="ExternalInput")
    # latent weights ship as bf16: sign() is invariant to bf16 rounding and
    # w4's bf16 cast is the same rounding the kernel would do on-device
    w1T = nc.dram_tensor("w1T", [F_IN, H], bf16, kind="ExternalInput")
    w2T = nc.dram_tensor("w2T", [H, H], bf16, kind="ExternalInput")
    w3T = nc.dram_tensor("w3T", [H, H], bf16, kind="ExternalInput")
    w4T = nc.dram_tensor("w4T", [H, N_OUT], bf16, kind="ExternalInput")
    # biases host-packed to per-partition layout: col 4*l+m = b{l+1}[m*128:(m+1)*128]
    ball = nc.dram_tensor("ball", [128, 12], f32, kind="ExternalInput")
    b4 = nc.dram_tensor("b4", [N_OUT, 1], f32, kind="ExternalInput")
    y = nc.dram_tensor("y", [N_OUT, b_shard], f32, kind="ExternalOutput")

    with tile.TileContext(nc) as tc:
        with (
            tc.tile_pool(name="wconst", bufs=1) as wpool,
            tc.tile_pool(name="wstage", bufs=2) as wstage,
            tc.tile_pool(name="xbf", bufs=4) as xbf_pool,
            tc.tile_pool(name="hbuf", bufs=8) as h_pool,
            tc.tile_pool(name="yout", bufs=4) as y_pool,
            tc.tile_pool(name="psum", bufs=8, space="PSUM") as ps_pool,
        ):
            # Sign bias: maps w==0 -> +1, matching where(w>=0,1,-1)
            sign_eps = wpool.tile([128, 1], f32, tag="sign_eps", name="sign_eps")
            nc.vector.memset(sign_eps[:], 1e-20)
            # dummy activation: pull the ACT table load off the critical path
            warm = wpool.tile([1, 1], bf16, tag="warm", name="warm")
            nc.scalar.activation(warm[:], sign_eps[0:1, :], ActFn.Sign, bias=0.0)

            def load_x(ci, splits=1):
                coff, cw = chunks[ci]
                csl = slice(coff, coff + cw)
                xb = xbf_pool.tile([K1, NK1, cw], bf16, tag="xb", name=f"xb{ci}")
                src = xT.ap()[:, csl].rearrange("(a p) n -> p a n", p=K1)
                bounds = [round(NK1 * s / splits) for s in range(splits + 1)]
                for s in range(splits):
                    k0, k1 = bounds[s], bounds[s + 1]
                    nc.sync.dma_start(xb[:, k0:k1, :], src[:, k0:k1, :])
                return xb

            # ---- weights: batched DMA + per-k-slice Sign on scalar engine.
            #      All loads share the sync HWDGE queue: dispatch order is the
            #      HBM priority order in the prologue. ----
            def prep_bin_load(w_dram, n_k, k_size, name, splits=1, eng=None):
                eng = eng or nc.sync
                wf = wstage.tile([k_size, n_k, H], bf16, tag="wstage", name=f"{name}f")
                src = w_dram.ap().rearrange("(a p) n -> p a n", p=k_size)
                bounds = [round(n_k * s / splits) for s in range(splits + 1)]
                for s in range(splits):
                    k0, k1 = bounds[s], bounds[s + 1]
                    eng.dma_start(wf[:, k0:k1, :], src[:, k0:k1, :])
                return wf

            def prep_bin_sign(wf, n_k, k_size, name):
                wb = wpool.tile([k_size, n_k, H], bf16, tag=name, name=name)
                for k in range(n_k):
                    nc.scalar.activation(
                        wb[:, k, :], wf[:, k, :], ActFn.Sign, bias=sign_eps[:k_size, :]
                    )
                return wb

            # prologue: tiny loads + w1 on the scalar HWDGE queue (fast, and
            # nothing slow sits ahead of the Sign ops); x chunks on sync —
            # the two dispatch streams run concurrently
            ballt = wpool.tile([128, 12], f32, tag="ballt", name="ballt")
            nc.scalar.dma_start(ballt[:], ball.ap()[:])
            b4t = wpool.tile([N_OUT, 1], f32, tag="b4t", name="b4t")
            nc.scalar.dma_start(b4t[:], b4.ap()[:])
            w1f = prep_bin_load(w1T, NK1, K1, "w1b", splits=2, eng=nc.scalar)
            xb0 = load_x(0)
            xb1 = load_x(1)
            w1b = prep_bin_sign(w1f, NK1, K1, "w1b")
            w2f = prep_bin_load(w2T, NKH, 128, "w2b")
            w2b = prep_bin_sign(w2f, NKH, 128, "w2b")
            w3f = prep_bin_load(w3T, NKH, 128, "w3b")
            w3b = prep_bin_sign(w3f, NKH, 128, "w3b")

            w4c = wpool.tile([128, NKH, N_OUT], bf16, tag="w4c", name="w4c")
            nc.sync.dma_start(
                w4c[:], w4T.ap().rearrange("(a p) n -> p a n", p=128)
            )

            b1t = ballt[:, 0:4]
            b2t = ballt[:, 4:8]
            b3t = ballt[:, 8:12]

            def layer(c, cw, ins_of_k, wtiles, btiles, n_k, name, k_outer=False):
                outs = []
                pss = [
                    ps_pool.tile([128, cw], f32, tag="ps", name=f"ps_{name}_{c}_{m}")
                    for m in range(NKH)
                ]
                # k_outer: emit k-round-robin across the 4 psum groups so the
                # PE has ready work as soon as the first k-slices land
                # (prologue only; steady state uses m-outer)
                order = (
                    [(m, k) for k in range(n_k) for m in range(NKH)]
                    if k_outer
                    else [(m, k) for m in range(NKH) for k in range(n_k)]
                )
                for m, k in order:
                    nc.tensor.matmul(
                        pss[m][:],
                        lhsT=wtiles[:, k, m * 128 : (m + 1) * 128],
                        rhs=ins_of_k(k),
                        start=(k == 0),
                        stop=(k == n_k - 1),
                    )
                for m in range(NKH):
                    ht = h_pool.tile(
                        [128, cw], bf16, tag=f"h{name}", name=f"h{name}_{c}_{m}"
                    )
                    nc.scalar.activation(
                        ht[:], pss[m][:], ActFn.Relu, bias=btiles[:, m : m + 1], scale=1.0
                    )
                    outs.append(ht)
                return outs

            def head(c, coff, cw, h3):
                ps4 = ps_pool.tile([N_OUT, cw], f32, tag="ps", name=f"ps4_{c}")
                for k in range(NKH):
                    nc.tensor.matmul(
                        ps4[:],
                        lhsT=w4c[:, k, :],
                        rhs=h3[k][:],
                        start=(k == 0),
                        stop=(k == NKH - 1),
                    )
                yt = y_pool.tile([N_OUT, cw], f32, tag="yt", name=f"yt_{c}")
                nc.scalar.activation(
                    yt[:], ps4[:], ActFn.Identity, bias=b4t[:], scale=1.0
                )
                nc.sync.dma_start(y.ap()[:, coff : coff + cw], yt[:])

            # ---- main loop: interleaved chunk pairs (last may be solo) ----
            def do_pair(pair, first=False):
                xbs = []
                for i, ci in enumerate(pair):
                    if first:
                        xbs.append(xb0 if i == 0 else xb1)
                    else:
                        xbs.append(load_x(ci))
                hs = []
                for i, ci in enumerate(pair):
                    xb = xbs[i]
                    hs.append(
                        layer(
                            ci,
                            chunks[ci][1],
                            lambda k, xb=xb: xb[:, k, :],
                            w1b,
                            b1t,
                            NK1,
                            "1",
                            k_outer=first and i == 0,
                        )
                    )
                for name, wb, bt in (("2", w2b, b2t), ("3", w3b, b3t)):
                    hs = [
                        layer(
                            ci,
                            chunks[ci][1],
                            lambda k, h=hs[i]: h[k][:],
                            wb,
                            bt,
                            NKH,
                            name,
                        )
                        for i, ci in enumerate(pair)
                    ]
                for i, ci in enumerate(pair):
                    head(ci, chunks[ci][0], chunks[ci][1], hs[i])

            pairs = [
                list(range(s, min(s + 2, len(chunks))))
                for s in range(0, len(chunks), 2)
            ]
            for pi, pair in enumerate(pairs):
                do_pair(pair, first=(pi == 0))

    nc.compile()
    return nc


_CACHE = {}


def _get_nc(b_shard: int):
    key = b_shard
    if key not in _CACHE:
        _CACHE[key] = build_nc(b_shard)
    return _CACHE[key]


def make_in_maps(x, w1, b1, w2, b2, w3, b3, w4, b4, n_cores=N_CORES):
    """Host-side layout prep (slicing/transpose/dtype marshalling only)."""
    import ml_dtypes

    B = x.shape[0]
    b_shard = B // n_cores
    xT = np.ascontiguousarray(
        np.asarray(x, dtype=np.float32).T.astype(ml_dtypes.bfloat16)
    )
    ball = np.concatenate(
        [np.asarray(b, np.float32).reshape(NKH, 128).T for b in (b1, b2, b3)], axis=1
    )
    def wprep(w):
        return np.ascontiguousarray(
            np.asarray(w, np.float32).T.astype(ml_dtypes.bfloat16)
        )

    common = {
        "w1T": wprep(w1),
        "w2T": wprep(w2),
        "w3T": wprep(w3),
        "w4T": wprep(w4),
        "ball": np.ascontiguousarray(ball),
        "b4": np.asarray(b4, np.float32).reshape(N_OUT, 1),
    }
    return [
        {"xT": np.ascontiguousarray(xT[:, i * b_shard : (i + 1) * b_shard]), **common}
        for i in range(n_cores)
    ]


def kernel(x, w1, b1, w2, b2, w3, b3, w4, b4):
    from concourse.bass_utils import run_bass_kernel_spmd

    B = x.shape[0]
    b_shard = B // N_CORES
    nc = _get_nc(b_shard)
    in_maps = make_in_maps(x, w1, b1, w2, b2, w3, b3, w4, b4)
    res = run_bass_kernel_spmd(nc, in_maps, core_ids=list(range(N_CORES)))
    yT = np.concatenate([res.results[i]["y"] for i in range(N_CORES)], axis=1)
    return np.ascontiguousarray(yT.T).astype(np.float32)


# revision 31
# speedup vs baseline: 1.2017x; 1.2017x over previous
"""Trainium2 Bass kernel for BinaryMLP:
    h = relu(x @ sign(w1).T + b1); h = relu(h @ sign(w2).T + b2);
    h = relu(h @ sign(w3).T + b3); y = h @ w4.T + b4

Data-parallel over 8 NeuronCores: batch 65536 -> 8192 rows/core, weights
replicated. On-device dataflow is feature-major ("transposed"): activations
live in SBUF as [feature_partition, batch_free] so every layer's contraction
dim (the feature/hidden dim) is the PE partition dim. The host only slices
the batch, transposes/casts for layout, and concatenates the result back.

Compute is bf16 on the tensor engine (binary +-1 weights are exact in bf16;
PSUM accumulates fp32; x is rounded to bf16 host-side — identical numerics
to an on-device cast). Binarization (sign of the latent fp32 weights) runs
on the scalar engine; bias+relu runs on the scalar engine reading PSUM and
writing bf16 back to SBUF. Each batch chunk/weight matrix loads with one
batched DMA (packets spread over all 16 DMA engines); the sync-queue
dispatch order doubles as the HBM priority order in the prologue. Chunks
are processed in interleaved pairs so the PE can run chunk c+1's fc1 while
chunk c's relu results are still in flight (kills the layer-boundary
stalls; peak PSUM use stays at 8 banks).
"""

import numpy as np

N_CORES = 8
F_IN = 784  # input features: 7 k-tiles of 112
K1 = 112
NK1 = 7
H = 512  # hidden width: 4 k-tiles / m-tiles of 128
NKH = 4
N_OUT = 10
CHUNK = 512  # batch columns per moving-operand chunk


def build_nc(b_shard: int, num_devices: int = N_CORES, chunk: int = CHUNK):
    """Build + compile the per-core Bass program for a batch shard of
    b_shard columns. Every core runs the identical program."""
    import concourse.bacc as bacc
    import concourse.mybir as mybir
    import concourse.tile as tile

    f32 = mybir.dt.float32
    bf16 = mybir.dt.bfloat16
    ActFn = mybir.ActivationFunctionType

    # chunk schedule: first pair at half width so the PE can start on a
    # quarter of the data (same per-element PE efficiency at N=256), then
    # full-width chunks
    small = chunk // 2
    assert (b_shard - 2 * small) % chunk == 0
    chunks = [(0, small), (small, small)]
    off = 2 * small
    while off < b_shard:
        chunks.append((off, chunk))
        off += chunk

    nc = bacc.Bacc(
        "TRN2", target_bir_lowering=False, debug=False, num_devices=num_devices
    )

    xT = nc.dram_tensor("xT", [F_IN, b_shard], bf16, kind="ExternalInput")
    # latent weights ship as bf16: sign() is invariant to bf16 rounding and
    # w4's bf16 cast is the same rounding the kernel would do on-device
    w1T = nc.dram_tensor("w1T", [F_IN, H], bf16, kind="ExternalInput")
    w2T = nc.dram_tensor("w2T", [H, H], bf16, kind="ExternalInput")
    w3T = nc.dram_tensor("w3T", [H, H], bf16, kind="ExternalInput")
    w4T = nc.dram_tensor("w4T", [H, N_OUT], bf16, kind="ExternalInput")
    # biases host-packed to per-partition layout: col 4*l+m = b{l+1}[m*128:(m+1)*128]
    ball = nc.dram_tensor("ball", [128, 12], f32, kind="ExternalInput")
    b4 = nc.dram_tensor("b4", [N_OUT, 1], f32, kind="ExternalInput")
    y = nc.dram_tensor("y", [N_OUT, b_shard], f32, kind="ExternalOutput")

    with tile.TileContext(nc) as tc:
        with (
            tc.tile_pool(name="wconst", bufs=1) as wpool,
            tc.tile_pool(name="wstage", bufs=2) as wstage,
            tc.tile_pool(name="xbf", bufs=4) as xbf_pool,
            tc.tile_pool(name="hbuf", bufs=8) as h_pool,
            tc.tile_pool(name="yout", bufs=4) as y_pool,
            tc.tile_pool(name="psum", bufs=8, space="PSUM") as ps_pool,
        ):
            # Sign bias: maps w==0 -> +1, matching where(w>=0,1,-1)
            sign_eps = wpool.tile([128, 1], f32, tag="sign_eps", name="sign_eps")
            nc.vector.memset(sign_eps[:], 1e-20)
            # dummy activation: pull the ACT table load off the critical path
            warm = wpool.tile([1, 1], bf16, tag="warm", name="warm")
            nc.scalar.activation(warm[:], sign_eps[0:1, :], ActFn.Sign, bias=0.0)

            def load_x(ci, splits=1, after=None):
                coff, cw = chunks[ci]
                csl = slice(coff, coff + cw)
                xb = xbf_pool.tile([K1, NK1, cw], bf16, tag="xb", name=f"xb{ci}")
                src = xT.ap()[:, csl].rearrange("(a p) n -> p a n", p=K1)
                bounds = [round(NK1 * s / splits) for s in range(splits + 1)]
                for s in range(splits):
                    k0, k1 = bounds[s], bounds[s + 1]
                    dma = nc.sync.dma_start(xb[:, k0:k1, :], src[:, k0:k1, :])
                    if after is not None:
                        tile.add_dep_helper(dma.ins, after.ins, sync=True)
                return xb

            # ---- weights: batched DMA + per-k-slice Sign on scalar engine.
            #      All loads share the sync HWDGE queue: dispatch order is the
            #      HBM priority order in the prologue. ----
            def prep_bin_load(w_dram, n_k, k_size, name, splits=1, after=None):
                wf = wstage.tile([k_size, n_k, H], bf16, tag="wstage", name=f"{name}f")
                src = w_dram.ap().rearrange("(a p) n -> p a n", p=k_size)
                bounds = [round(n_k * s / splits) for s in range(splits + 1)]
                for s in range(splits):
                    k0, k1 = bounds[s], bounds[s + 1]
                    dma = nc.sync.dma_start(wf[:, k0:k1, :], src[:, k0:k1, :])
                    if after is not None:
                        tile.add_dep_helper(dma.ins, after.ins, sync=True)
                return wf

            def prep_bin_sign(wf, n_k, k_size, name):
                wb = wpool.tile([k_size, n_k, H], bf16, tag=name, name=name)
                for k in range(n_k):
                    nc.scalar.activation(
                        wb[:, k, :], wf[:, k, :], ActFn.Sign, bias=sign_eps[:k_size, :]
                    )
                return wb

            # prologue: w1 first on the sync ring (its packets get HBM
            # priority), then x0/x1; tiny bias loads on the scalar queue.
            # w2/w3/w4/next-pair-x dispatches are dep-anchored behind early
            # fc1 matmuls so their packets don't queue ahead of w1/x0.
            ballt = wpool.tile([128, 12], f32, tag="ballt", name="ballt")
            nc.scalar.dma_start(ballt[:], ball.ap()[:])
            b4t = wpool.tile([N_OUT, 1], f32, tag="b4t", name="b4t")
            nc.scalar.dma_start(b4t[:], b4.ap()[:])
            w1f = prep_bin_load(w1T, NK1, K1, "w1b", splits=2)
            xb0 = load_x(0)
            xb1 = load_x(1)
            w1b = prep_bin_sign(w1f, NK1, K1, "w1b")

            b1t = ballt[:, 0:4]
            b2t = ballt[:, 4:8]
            b3t = ballt[:, 8:12]

            def layer(c, cw, ins_of_k, wtiles, btiles, n_k, name, k_outer=False):
                outs = []
                mms = []
                pss = [
                    ps_pool.tile([128, cw], f32, tag="ps", name=f"ps_{name}_{c}_{m}")
                    for m in range(NKH)
                ]
                # k_outer: emit k-round-robin across the 4 psum groups so the
                # PE has ready work as soon as the first k-slices land
                # (prologue only; steady state uses m-outer)
                order = (
                    [(m, k) for k in range(n_k) for m in range(NKH)]
                    if k_outer
                    else [(m, k) for m in range(NKH) for k in range(n_k)]
                )
                for m, k in order:
                    mms.append(
                        nc.tensor.matmul(
                            pss[m][:],
                            lhsT=wtiles[:, k, m * 128 : (m + 1) * 128],
                            rhs=ins_of_k(k),
                            start=(k == 0),
                            stop=(k == n_k - 1),
                        )
                    )
                for m in range(NKH):
                    ht = h_pool.tile(
                        [128, cw], bf16, tag=f"h{name}", name=f"h{name}_{c}_{m}"
                    )
                    nc.scalar.activation(
                        ht[:], pss[m][:], ActFn.Relu, bias=btiles[:, m : m + 1], scale=1.0
                    )
                    outs.append(ht)
                return mms, outs

            def head(c, coff, cw, h3):
                ps4 = ps_pool.tile([N_OUT, cw], f32, tag="ps", name=f"ps4_{c}")
                for k in range(NKH):
                    nc.tensor.matmul(
                        ps4[:],
                        lhsT=w4c[:, k, :],
                        rhs=h3[k][:],
                        start=(k == 0),
                        stop=(k == NKH - 1),
                    )
                yt = y_pool.tile([N_OUT, cw], f32, tag="yt", name=f"yt_{c}")
                nc.scalar.activation(
                    yt[:], ps4[:], ActFn.Identity, bias=b4t[:], scale=1.0
                )
                nc.sync.dma_start(y.ap()[:, coff : coff + cw], yt[:])

            # ---- pair 0: weight prep interleaved with the layer flow so
            # later loads' packets queue behind what's needed first ----
            cwA, cwB = chunks[0][1], chunks[1][1]
            mmsA, h1A = layer(
                0, cwA, lambda k: xb0[:, k, :], w1b, b1t, NK1, "1", k_outer=True
            )
            mmsB, h1B = layer(1, cwB, lambda k: xb1[:, k, :], w1b, b1t, NK1, "1")

            w2f = prep_bin_load(w2T, NKH, 128, "w2b", after=mmsA[0])
            w2b = prep_bin_sign(w2f, NKH, 128, "w2b")
            _, h2A = layer(0, cwA, lambda k: h1A[k][:], w2b, b2t, NKH, "2")
            _, h2B = layer(1, cwB, lambda k: h1B[k][:], w2b, b2t, NKH, "2")

            w3f = prep_bin_load(w3T, NKH, 128, "w3b", after=mmsB[0])
            w3b = prep_bin_sign(w3f, NKH, 128, "w3b")
            _, h3A = layer(0, cwA, lambda k: h2A[k][:], w3b, b3t, NKH, "3")
            _, h3B = layer(1, cwB, lambda k: h2B[k][:], w3b, b3t, NKH, "3")

            w4c = wpool.tile([128, NKH, N_OUT], bf16, tag="w4c", name="w4c")
            w4dma = nc.sync.dma_start(
                w4c[:], w4T.ap().rearrange("(a p) n -> p a n", p=128)
            )
            tile.add_dep_helper(w4dma.ins, mmsB[0].ins, sync=True)
            head(0, chunks[0][0], cwA, h3A)
            head(1, chunks[1][0], cwB, h3B)

            # ---- remaining pairs ----
            def do_pair(pair, after=None):
                xbs = [load_x(ci, after=after) for ci in pair]
                hs = []
                for i, ci in enumerate(pair):
                    xb = xbs[i]
                    _, outs = layer(
                        ci, chunks[ci][1], lambda k, xb=xb: xb[:, k, :], w1b, b1t,
                        NK1, "1",
                    )
                    hs.append(outs)
                for name, wb, bt in (("2", w2b, b2t), ("3", w3b, b3t)):
                    hs = [
                        layer(
                            ci, chunks[ci][1], lambda k, h=hs[i]: h[k][:], wb, bt,
                            NKH, name,
                        )[1]
                        for i, ci in enumerate(pair)
                    ]
                for i, ci in enumerate(pair):
                    head(ci, chunks[ci][0], chunks[ci][1], hs[i])

            pairs = [
                list(range(s, min(s + 2, len(chunks))))
                for s in range(2, len(chunks), 2)
            ]
            for pi, pair in enumerate(pairs):
                do_pair(pair, after=mmsB[0] if pi == 0 else None)

    nc.compile()
    return nc


_CACHE = {}


def _get_nc(b_shard: int):
    key = b_shard
    if key not in _CACHE:
        _CACHE[key] = build_nc(b_shard)
    return _CACHE[key]


def make_in_maps(x, w1, b1, w2, b2, w3, b3, w4, b4, n_cores=N_CORES):
    """Host-side layout prep (slicing/transpose/dtype marshalling only)."""
    import ml_dtypes

    B = x.shape[0]
    b_shard = B // n_cores
    xT = np.ascontiguousarray(
        np.asarray(x, dtype=np.float32).T.astype(ml_dtypes.bfloat16)
    )
    ball = np.concatenate(
        [np.asarray(b, np.float32).reshape(NKH, 128).T for b in (b1, b2, b3)], axis=1
    )
    def wprep(w):
        return np.ascontiguousarray(
            np.asarray(w, np.float32).T.astype(ml_dtypes.bfloat16)
        )

    common = {
        "w1T": wprep(w1),
        "w2T": wprep(w2),
        "w3T": wprep(w3),
        "w4T": wprep(w4),
        "ball": np.ascontiguousarray(ball),
        "b4": np.asarray(b4, np.float32).reshape(N_OUT, 1),
    }
    return [
        {"xT": np.ascontiguousarray(xT[:, i * b_shard : (i + 1) * b_shard]), **common}
        for i in range(n_cores)
    ]


def kernel(x, w1, b1, w2, b2, w3, b3, w4, b4):
    from concourse.bass_utils import run_bass_kernel_spmd

    B = x.shape[0]
    b_shard = B // N_CORES
    nc = _get_nc(b_shard)
    in_maps = make_in_maps(x, w1, b1, w2, b2, w3, b3, w4, b4)
    res = run_bass_kernel_spmd(nc, in_maps, core_ids=list(range(N_CORES)))
    yT = np.concatenate([res.results[i]["y"] for i in range(N_CORES)], axis=1)
    return np.ascontiguousarray(yT.T).astype(np.float32)


# revision 33
# speedup vs baseline: 1.2027x; 1.0009x over previous
"""Trainium2 Bass kernel for BinaryMLP:
    h = relu(x @ sign(w1).T + b1); h = relu(h @ sign(w2).T + b2);
    h = relu(h @ sign(w3).T + b3); y = h @ w4.T + b4

Data-parallel over 8 NeuronCores: batch 65536 -> 8192 rows/core, weights
replicated. On-device dataflow is feature-major ("transposed"): activations
live in SBUF as [feature_partition, batch_free] so every layer's contraction
dim (the feature/hidden dim) is the PE partition dim. The host only slices
the batch, transposes/casts for layout, and concatenates the result back.

Compute is bf16 on the tensor engine (binary +-1 weights are exact in bf16;
PSUM accumulates fp32; x is rounded to bf16 host-side — identical numerics
to an on-device cast). Binarization (sign of the latent fp32 weights) runs
on the scalar engine; bias+relu runs on the scalar engine reading PSUM and
writing bf16 back to SBUF. Each batch chunk/weight matrix loads with one
batched DMA (packets spread over all 16 DMA engines); the sync-queue
dispatch order doubles as the HBM priority order in the prologue. Chunks
are processed in interleaved pairs so the PE can run chunk c+1's fc1 while
chunk c's relu results are still in flight (kills the layer-boundary
stalls; peak PSUM use stays at 8 banks).
"""

import numpy as np

N_CORES = 8
F_IN = 784  # input features: 7 k-tiles of 112
K1 = 112
NK1 = 7
H = 512  # hidden width: 4 k-tiles / m-tiles of 128
NKH = 4
N_OUT = 10
CHUNK = 512  # batch columns per moving-operand chunk


def build_nc(b_shard: int, num_devices: int = N_CORES, chunk: int = CHUNK):
    """Build + compile the per-core Bass program for a batch shard of
    b_shard columns. Every core runs the identical program."""
    import concourse.bacc as bacc
    import concourse.mybir as mybir
    import concourse.tile as tile

    f32 = mybir.dt.float32
    bf16 = mybir.dt.bfloat16
    ActFn = mybir.ActivationFunctionType

    # chunk schedule: first pair at half width so the PE can start on a
    # quarter of the data (same per-element PE efficiency at N=256), then
    # full-width chunks
    small = chunk // 2
    assert (b_shard - 2 * small) % chunk == 0
    chunks = [(0, small), (small, small)]
    off = 2 * small
    while off < b_shard:
        chunks.append((off, chunk))
        off += chunk

    nc = bacc.Bacc(
        "TRN2", target_bir_lowering=False, debug=False, num_devices=num_devices
    )

    xT = nc.dram_tensor("xT", [F_IN, b_shard], bf16, kind="ExternalInput")
    # latent weights ship as bf16: sign() is invariant to bf16 rounding and
    # w4's bf16 cast is the same rounding the kernel would do on-device
    w1T = nc.dram_tensor("w1T", [F_IN, H], bf16, kind="ExternalInput")
    w2T = nc.dram_tensor("w2T", [H, H], bf16, kind="ExternalInput")
    w3T = nc.dram_tensor("w3T", [H, H], bf16, kind="ExternalInput")
    w4T = nc.dram_tensor("w4T", [H, N_OUT], bf16, kind="ExternalInput")
    # biases host-packed to per-partition layout: col 4*l+m = b{l+1}[m*128:(m+1)*128]
    ball = nc.dram_tensor("ball", [128, 12], f32, kind="ExternalInput")
    b4 = nc.dram_tensor("b4", [N_OUT, 1], f32, kind="ExternalInput")
    y = nc.dram_tensor("y", [N_OUT, b_shard], f32, kind="ExternalOutput")

    with tile.TileContext(nc) as tc:
        with (
            tc.tile_pool(name="wconst", bufs=1) as wpool,
            tc.tile_pool(name="wstage", bufs=2) as wstage,
            tc.tile_pool(name="xbf", bufs=4) as xbf_pool,
            tc.tile_pool(name="hbuf", bufs=8) as h_pool,
            tc.tile_pool(name="yout", bufs=4) as y_pool,
            tc.tile_pool(name="psum", bufs=8, space="PSUM") as ps_pool,
        ):
            # Sign bias: maps w==0 -> +1, matching where(w>=0,1,-1)
            sign_eps = wpool.tile([128, 1], f32, tag="sign_eps", name="sign_eps")
            nc.vector.memset(sign_eps[:], 1e-20)
            # dummy activation: pull the ACT table load off the critical path
            warm = wpool.tile([1, 1], bf16, tag="warm", name="warm")
            nc.scalar.activation(warm[:], sign_eps[0:1, :], ActFn.Sign, bias=0.0)
            # PE warm-up: keep the PE busy while the prologue DMAs stream so
            # the HAM clock gate is at 8/8 (2.4 GHz) when real matmuls start
            pe_seed = wpool.tile([1, 64], bf16, tag="pe_seed", name="pe_seed")
            nc.vector.memset(pe_seed[:], 1.0)
            pe_sink = ps_pool.tile([2, 64], f32, tag="ps", name="pe_sink")
            for _ in range(110):
                nc.tensor.matmul(
                    pe_sink[:], lhsT=pe_seed[:, 0:2], rhs=pe_seed[:],
                    start=True, stop=True,
                )

            def load_x(ci, splits=1, after=None):
                coff, cw = chunks[ci]
                csl = slice(coff, coff + cw)
                xb = xbf_pool.tile([K1, NK1, cw], bf16, tag="xb", name=f"xb{ci}")
                src = xT.ap()[:, csl].rearrange("(a p) n -> p a n", p=K1)
                bounds = [round(NK1 * s / splits) for s in range(splits + 1)]
                for s in range(splits):
                    k0, k1 = bounds[s], bounds[s + 1]
                    dma = nc.sync.dma_start(xb[:, k0:k1, :], src[:, k0:k1, :])
                    if after is not None:
                        tile.add_dep_helper(dma.ins, after.ins, sync=True)
                return xb

            # ---- weights: batched DMA + per-k-slice Sign on scalar engine.
            #      All loads share the sync HWDGE queue: dispatch order is the
            #      HBM priority order in the prologue. ----
            def prep_bin_load(w_dram, n_k, k_size, name, splits=1, after=None):
                wf = wstage.tile([k_size, n_k, H], bf16, tag="wstage", name=f"{name}f")
                src = w_dram.ap().rearrange("(a p) n -> p a n", p=k_size)
                bounds = [round(n_k * s / splits) for s in range(splits + 1)]
                for s in range(splits):
                    k0, k1 = bounds[s], bounds[s + 1]
                    dma = nc.sync.dma_start(wf[:, k0:k1, :], src[:, k0:k1, :])
                    if after is not None:
                        tile.add_dep_helper(dma.ins, after.ins, sync=True)
                return wf

            def prep_bin_sign(wf, n_k, k_size, name):
                wb = wpool.tile([k_size, n_k, H], bf16, tag=name, name=name)
                for k in range(n_k):
                    nc.scalar.activation(
                        wb[:, k, :], wf[:, k, :], ActFn.Sign, bias=sign_eps[:k_size, :]
                    )
                return wb

            # prologue: w1 first on the sync ring (its packets get HBM
            # priority), then x0/x1; tiny bias loads on the scalar queue.
            # w2/w3/w4/next-pair-x dispatches are dep-anchored behind early
            # fc1 matmuls so their packets don't queue ahead of w1/x0.
            ballt = wpool.tile([128, 12], f32, tag="ballt", name="ballt")
            nc.scalar.dma_start(ballt[:], ball.ap()[:])
            b4t = wpool.tile([N_OUT, 1], f32, tag="b4t", name="b4t")
            nc.scalar.dma_start(b4t[:], b4.ap()[:])
            w1f = prep_bin_load(w1T, NK1, K1, "w1b", splits=2)
            xb0 = load_x(0)
            xb1 = load_x(1)
            w1b = prep_bin_sign(w1f, NK1, K1, "w1b")

            b1t = ballt[:, 0:4]
            b2t = ballt[:, 4:8]
            b3t = ballt[:, 8:12]

            def layer(c, cw, ins_of_k, wtiles, btiles, n_k, name, k_outer=False):
                outs = []
                mms = []
                pss = [
                    ps_pool.tile([128, cw], f32, tag="ps", name=f"ps_{name}_{c}_{m}")
                    for m in range(NKH)
                ]
                # k_outer: emit k-round-robin across the 4 psum groups so the
                # PE has ready work as soon as the first k-slices land
                # (prologue only; steady state uses m-outer)
                order = (
                    [(m, k) for k in range(n_k) for m in range(NKH)]
                    if k_outer
                    else [(m, k) for m in range(NKH) for k in range(n_k)]
                )
                for m, k in order:
                    mms.append(
                        nc.tensor.matmul(
                            pss[m][:],
                            lhsT=wtiles[:, k, m * 128 : (m + 1) * 128],
                            rhs=ins_of_k(k),
                            start=(k == 0),
                            stop=(k == n_k - 1),
                        )
                    )
                for m in range(NKH):
                    ht = h_pool.tile(
                        [128, cw], bf16, tag=f"h{name}", name=f"h{name}_{c}_{m}"
                    )
                    nc.scalar.activation(
                        ht[:], pss[m][:], ActFn.Relu, bias=btiles[:, m : m + 1], scale=1.0
                    )
                    outs.append(ht)
                return mms, outs

            def head(c, coff, cw, h3):
                ps4 = ps_pool.tile([N_OUT, cw], f32, tag="ps", name=f"ps4_{c}")
                for k in range(NKH):
                    nc.tensor.matmul(
                        ps4[:],
                        lhsT=w4c[:, k, :],
                        rhs=h3[k][:],
                        start=(k == 0),
                        stop=(k == NKH - 1),
                    )
                yt = y_pool.tile([N_OUT, cw], f32, tag="yt", name=f"yt_{c}")
                nc.scalar.activation(
                    yt[:], ps4[:], ActFn.Identity, bias=b4t[:], scale=1.0
                )
                nc.sync.dma_start(y.ap()[:, coff : coff + cw], yt[:])

            # ---- pair 0: weight prep interleaved with the layer flow so
            # later loads' packets queue behind what's needed first ----
            cwA, cwB = chunks[0][1], chunks[1][1]
            mmsA, h1A = layer(
                0, cwA, lambda k: xb0[:, k, :], w1b, b1t, NK1, "1", k_outer=True
            )
            mmsB, h1B = layer(1, cwB, lambda k: xb1[:, k, :], w1b, b1t, NK1, "1")

            w2f = prep_bin_load(w2T, NKH, 128, "w2b", after=mmsA[0])
            w2b = prep_bin_sign(w2f, NKH, 128, "w2b")
            _, h2A = layer(0, cwA, lambda k: h1A[k][:], w2b, b2t, NKH, "2")
            _, h2B = layer(1, cwB, lambda k: h1B[k][:], w2b, b2t, NKH, "2")

            w3f = prep_bin_load(w3T, NKH, 128, "w3b", after=mmsB[0])
            w3b = prep_bin_sign(w3f, NKH, 128, "w3b")
            _, h3A = layer(0, cwA, lambda k: h2A[k][:], w3b, b3t, NKH, "3")
            _, h3B = layer(1, cwB, lambda k: h2B[k][:], w3b, b3t, NKH, "3")

            w4c = wpool.tile([128, NKH, N_OUT], bf16, tag="w4c", name="w4c")
            w4dma = nc.sync.dma_start(
                w4c[:], w4T.ap().rearrange("(a p) n -> p a n", p=128)
            )
            tile.add_dep_helper(w4dma.ins, mmsB[0].ins, sync=True)
            head(0, chunks[0][0], cwA, h3A)
            head(1, chunks[1][0], cwB, h3B)

            # ---- remaining pairs ----
            def do_pair(pair, after=None):
                xbs = [load_x(ci, after=after) for ci in pair]
                hs = []
                for i, ci in enumerate(pair):
                    xb = xbs[i]
                    _, outs = layer(
                        ci, chunks[ci][1], lambda k, xb=xb: xb[:, k, :], w1b, b1t,
                        NK1, "1",
                    )
                    hs.append(outs)
                for name, wb, bt in (("2", w2b, b2t), ("3", w3b, b3t)):
                    hs = [
                        layer(
                            ci, chunks[ci][1], lambda k, h=hs[i]: h[k][:], wb, bt,
                            NKH, name,
                        )[1]
                        for i, ci in enumerate(pair)
                    ]
                for i, ci in enumerate(pair):
                    head(ci, chunks[ci][0], chunks[ci][1], hs[i])

            pairs = [
                list(range(s, min(s + 2, len(chunks))))
                for s in range(2, len(chunks), 2)
            ]
            for pi, pair in enumerate(pairs):
                do_pair(pair, after=mmsB[0] if pi == 0 else None)

    nc.compile()
    return nc


_CACHE = {}


def _get_nc(b_shard: int):
    key = b_shard
    if key not in _CACHE:
        _CACHE[key] = build_nc(b_shard)
    return _CACHE[key]


def make_in_maps(x, w1, b1, w2, b2, w3, b3, w4, b4, n_cores=N_CORES):
    """Host-side layout prep (slicing/transpose/dtype marshalling only)."""
    import ml_dtypes

    B = x.shape[0]
    b_shard = B // n_cores
    xT = np.ascontiguousarray(
        np.asarray(x, dtype=np.float32).T.astype(ml_dtypes.bfloat16)
    )
    ball = np.concatenate(
        [np.asarray(b, np.float32).reshape(NKH, 128).T for b in (b1, b2, b3)], axis=1
    )
    def wprep(w):
        return np.ascontiguousarray(
            np.asarray(w, np.float32).T.astype(ml_dtypes.bfloat16)
        )

    common = {
        "w1T": wprep(w1),
        "w2T": wprep(w2),
        "w3T": wprep(w3),
        "w4T": wprep(w4),
        "ball": np.ascontiguousarray(ball),
        "b4": np.asarray(b4, np.float32).reshape(N_OUT, 1),
    }
    return [
        {"xT": np.ascontiguousarray(xT[:, i * b_shard : (i + 1) * b_shard]), **common}
        for i in range(n_cores)
    ]


def kernel(x, w1, b1, w2, b2, w3, b3, w4, b4):
    from concourse.bass_utils import run_bass_kernel_spmd

    B = x.shape[0]
    b_shard = B // N_CORES
    nc = _get_nc(b_shard)
    in_maps = make_in_maps(x, w1, b1, w2, b2, w3, b3, w4, b4)
    res = run_bass_kernel_spmd(nc, in_maps, core_ids=list(range(N_CORES)))
    yT = np.concatenate([res.results[i]["y"] for i in range(N_CORES)], axis=1)
    return np.ascontiguousarray(yT.T).astype(np.float32)
